# revision 4
# baseline (speedup 1.0000x reference)
"""Contrastive loss kernel for Trainium2, 8 NeuronCores (SPMD).

Math (matches the reference):
    z = concat(normalize(z_i), normalize(z_j))        # (2B, D) = (8192, 256)
    sim = (z @ z.T) / T
    positives[g] = sim[g, (g+B) mod 2B]               # (2B,)
    neg_max[g] = max_{j != g} sim[g, j]
    loss = mean(neg_max) - logsumexp(positives)       # scalar (note sign algebra
                                                      # of the reference collapses
                                                      # to exactly this)

Sharding: data-parallel over rows. Core k receives z rolled by -1024*k so its
band is always rows [0, 1024) of its local copy -> identical static program on
every core (diagonal / positive blocks land at fixed tile offsets).

Device pipeline per core:
  DMA load z (f32, row-major) -> ACT square+accum row norms -> sqrt + recip ->
  ACT scale+downcast to normalized bf16 -> PE transpose to [d, row] ->
  bf16 matmuls (sim tiles in PSUM f32) -> DVE row-max (self-sim masked by
  subtracting BIG*I), positives extracted via identity-dot ->
  outputs row_max (128, 8) and pos (128, 8) per core.

Host: gather, divide by T, mean/LSE in float64, return float32 scalar.
"""

import numpy as np

TEMPERATURE = 0.1
B, D = 4096, 256
R = 2 * B                # 8192 total rows
NCORES = 8
MROWS = R // NCORES      # 1024 rows per core
P = 128                  # SBUF partitions
NT_ROW = R // P          # 64 row tiles of (128, 256)
MB = MROWS // P          # 8 m-blocks per core
PAIR = 1024              # matmul moving-operand width (bf16) / psum tile width
NPAIR = R // PAIR        # 8 column-pairs
KC = D // P              # 2 contraction chunks of 128
BIG = 30000.0            # diag mask subtrahend (cos <= 1)

_CACHE = {}


def _build_nc():
    from contextlib import ExitStack

    import concourse.bass as bass
    import concourse.mybir as mybir
    import concourse.tile as tile
    from concourse import bacc
    from concourse.masks import make_identity

    f32 = mybir.dt.float32
    bf16 = mybir.dt.bfloat16

    nc = bacc.Bacc(
        "TRN2",
        target_bir_lowering=False,
        debug=False,
        enable_asserts=False,
        num_devices=NCORES,
    )

    z_dram = nc.dram_tensor("z", [R, D], f32, kind="ExternalInput")
    rowmax_dram = nc.dram_tensor("row_max", [P, MB], f32, kind="ExternalOutput")
    pos_dram = nc.dram_tensor("pos", [P, MB], f32, kind="ExternalOutput")

    with tile.TileContext(nc) as tc, ExitStack() as ctx:
        singles = ctx.enter_context(tc.tile_pool(name="singles", bufs=1))
        big = ctx.enter_context(tc.tile_pool(name="big", bufs=1))
        sq_pool = ctx.enter_context(tc.tile_pool(name="sq_pool", bufs=3))
        scr_pool = ctx.enter_context(tc.tile_pool(name="scr_pool", bufs=2))
        psum_t = ctx.enter_context(
            tc.tile_pool(name="psum_t", bufs=2, space=bass.MemorySpace.PSUM)
        )
        psum_mm = ctx.enter_context(
            tc.tile_pool(name="psum_mm", bufs=3, space=bass.MemorySpace.PSUM)
        )

        # --- constants ---
        ident_bf = singles.tile([P, P], bf16)
        make_identity(nc, ident_bf)
        ident_f = singles.tile([P, P], f32)
        make_identity(nc, ident_f)
        bigI = singles.tile([P, P], f32)
        nc.gpsimd.memset(bigI, 0.0)
        nc.gpsimd.affine_select(
            out=bigI,
            in_=bigI,
            compare_op=mybir.AluOpType.not_equal,
            fill=BIG,
            base=0,
            pattern=[[-1, P]],
            channel_multiplier=1,
        )

        # --- persistent buffers ---
        z_rm = big.tile([P, NT_ROW, D], f32)       # row-major raw z
        znb = big.tile([P, NT_ROW, D], bf16)       # row-major normalized bf16
        zT0 = big.tile([P, R], bf16)               # [d 0:128, row]
        zT1 = big.tile([P, R], bf16)               # [d 128:256, row]
        zT = [zT0, zT1]
        n2 = singles.tile([P, NT_ROW], f32)        # row norms^2
        nrm = singles.tile([P, NT_ROW], f32)
        inv = singles.tile([P, NT_ROW], f32)       # 1/norm
        maxq = singles.tile([P, MB, NPAIR], f32)   # per-pair row maxes
        rowmax_sb = singles.tile([P, MB], f32)
        pos_sb = singles.tile([P, MB], f32)

        z_src = z_dram.ap().rearrange("(t p) d -> p t d", p=P)

        GROUPS = 8
        TPG = NT_ROW // GROUPS  # 8 row-tiles per DMA group
        for g in range(GROUPS):
            nc.sync.dma_start(
                out=z_rm[:, g * TPG : (g + 1) * TPG, :],
                in_=z_src[:, g * TPG : (g + 1) * TPG, :],
            )
            for j in range(TPG):
                t = g * TPG + j
                # row norm^2 via ACT square with free-dim accumulate
                sq_scr = sq_pool.tile([P, D], f32, name=f"sq_scr")
                nc.scalar.activation(
                    out=sq_scr,
                    in_=z_rm[:, t, :],
                    func=mybir.ActivationFunctionType.Square,
                    accum_out=n2[:, t : t + 1],
                )
            # inv = 1/sqrt(n2) for this group of 8 tiles
            gs = slice(g * TPG, (g + 1) * TPG)
            nc.scalar.activation(
                out=nrm[:, gs],
                in_=n2[:, gs],
                func=mybir.ActivationFunctionType.Sqrt,
            )
            nc.vector.reciprocal(out=inv[:, gs], in_=nrm[:, gs])
            for j in range(TPG):
                t = g * TPG + j
                # normalized bf16 row-major: znb = z * inv  (ACT copy w/ scale)
                nc.scalar.activation(
                    out=znb[:, t, :],
                    in_=z_rm[:, t, :],
                    func=mybir.ActivationFunctionType.Copy,
                    scale=inv[:, t : t + 1],
                )

        # --- transpose: znb (row-major) -> zT[c] ([d, row]) via PE ---
        for c in range(KC):
            for g in range(GROUPS):
                pt = psum_t.tile([P, PAIR], bf16, name="pt")
                for j in range(TPG):
                    t = g * TPG + j
                    nc.tensor.transpose(
                        out=pt[:, j * P : (j + 1) * P],
                        in_=znb[:, t, c * P : (c + 1) * P],
                        identity=ident_bf,
                    )
                nc.scalar.copy(
                    out=zT[c][:, g * PAIR : (g + 1) * PAIR],
                    in_=pt[:],
                )

        # --- main: sim row-band x all columns, row-max + positives ---
        for b in range(MB):
            o = b * P  # offset of the diag/positive 128-block inside its pair
            for q in range(NPAIR):
                pp = psum_mm.tile([P, PAIR], f32, name="pp")
                for c in range(KC):
                    for u in range(PAIR // 512):
                        nc.tensor.matmul(
                            pp[:, u * 512 : (u + 1) * 512],
                            zT[c][:, b * P : (b + 1) * P],
                            zT[c][:, q * PAIR + u * 512 : q * PAIR + (u + 1) * 512],
                            start=(c == 0),
                            stop=(c == KC - 1),
                        )
                if q == 4:
                    # positives: diag of the block at columns 4096+o .. +128
                    scr = scr_pool.tile([P, P], f32, name="scr")
                    nc.vector.tensor_mul(out=scr, in0=pp[:, o : o + P], in1=ident_f)
                    nc.vector.reduce_sum(
                        out=pos_sb[:, b : b + 1],
                        in_=scr,
                        axis=mybir.AxisListType.X,
                    )
                if q == 0:
                    # mask self-similarity on the diag block
                    nc.vector.tensor_sub(
                        out=pp[:, o : o + P],
                        in0=pp[:, o : o + P],
                        in1=bigI,
                    )
                nc.vector.reduce_max(
                    out=maxq[:, b, q : q + 1],
                    in_=pp[:],
                    axis=mybir.AxisListType.X,
                )
            nc.vector.reduce_max(
                out=rowmax_sb[:, b : b + 1],
                in_=maxq[:, b, :],
                axis=mybir.AxisListType.X,
            )

        nc.sync.dma_start(out=rowmax_dram.ap(), in_=rowmax_sb[:])
        nc.sync.dma_start(out=pos_dram.ap(), in_=pos_sb[:])

    nc.compile()
    return nc


def _get_nc():
    if "nc" not in _CACHE:
        _CACHE["nc"] = _build_nc()
    return _CACHE["nc"]


def _finish(rowmax_all: np.ndarray, pos_all: np.ndarray) -> np.ndarray:
    negmax = rowmax_all.astype(np.float64) / TEMPERATURE
    pos = pos_all.astype(np.float64) / TEMPERATURE
    m = pos.max()
    lse = np.log(np.exp(pos - m).sum()) + m
    return np.array(negmax.mean() - lse, dtype=np.float32)


def kernel(z_i: np.ndarray, z_j: np.ndarray, _collect=None, _run_kwargs=None) -> np.ndarray:
    from concourse.bass_utils import run_bass_kernel_spmd

    z_full = np.concatenate(
        [np.asarray(z_i, np.float32), np.asarray(z_j, np.float32)], axis=0
    )
    in_maps = [
        {"z": np.ascontiguousarray(np.roll(z_full, -k * MROWS, axis=0))}
        for k in range(NCORES)
    ]
    nc = _get_nc()
    res = run_bass_kernel_spmd(
        nc, in_maps, core_ids=list(range(NCORES)), **(_run_kwargs or {})
    )
    if _collect is not None:
        _collect.append(res)
    rowmax_all = np.concatenate(
        [r["row_max"].T.reshape(-1) for r in res.results]
    )  # (8192,) in original row order
    pos_all = np.concatenate([r["pos"].T.reshape(-1) for r in res.results])
    return _finish(rowmax_all, pos_all)


# revision 6
# speedup vs baseline: 1.3177x; 1.3177x over previous
"""Contrastive loss kernel for Trainium2, 8 NeuronCores (SPMD).

Math (matches the reference):
    z = concat(normalize(z_i), normalize(z_j))        # (2B, D) = (8192, 256)
    sim = (z @ z.T) / T
    positives[g] = sim[g, (g+B) mod 2B]               # (2B,)
    neg_max[g] = max_{j != g} sim[g, j]
    loss = mean(neg_max) - logsumexp(positives)       # scalar

Sharding: data-parallel over rows. Core k receives z rolled by -1024*k so its
band is always rows [0, 1024) of its local copy -> identical static program on
every core (diagonal / positive blocks land at fixed tile offsets).

Device pipeline per core (v3):
  gpsimd cast-DMA loads z as bf16 row-major -> ACT squares + DVE windowed
  reduce give row norms -> sqrt + recip -> in-place bf16 row scale (DVE
  tensor_scalar 4x) -> store normalized bf16 to DRAM scratch -> DMA xbar
  transpose loads build zT [d, row] (no compute engines involved) ->
  per 128-row block: 4 psum quads (8 matmuls each, weights reloaded once per
  contraction chunk), diag masked / positives extracted on psum, ACT evacuates
  each quad to a bf16 candidate row, DVE deep-folds the 8192-wide candidate
  (2x bf16 tensor_tensor max tree) to the block row-max.
Host: gather, divide by T, mean/LSE in float64, return float32 scalar.
"""

import numpy as np

TEMPERATURE = 0.1
B, D = 4096, 256
R = 2 * B                # 8192 total rows
NCORES = 8
MROWS = R // NCORES      # 1024 rows per core
P = 128                  # SBUF partitions
NT_ROW = R // P          # 64 row tiles of (128, 256)
MB = MROWS // P          # 8 m-blocks per core
QUAD = 2048              # psum quad width (4 banks)
NQ = R // QUAD           # 4 quads per block row
CH = 8                   # preprocessing chunks (8 row-tiles = 1024 rows each)
TPG = NT_ROW // CH
KC = D // P              # 2 contraction chunks of 128
BIG = 30000.0            # diag mask subtrahend (cos <= 1)

_CACHE = {}


def _build_nc():
    from contextlib import ExitStack

    import concourse.bass as bass
    import concourse.mybir as mybir
    import concourse.tile as tile
    from concourse import bacc
    from concourse.masks import make_identity

    f32 = mybir.dt.float32
    bf16 = mybir.dt.bfloat16
    AF = mybir.ActivationFunctionType
    X = mybir.AxisListType.X

    nc = bacc.Bacc(
        "TRN2",
        target_bir_lowering=False,
        debug=False,
        enable_asserts=False,
        num_devices=NCORES,
    )

    z_dram = nc.dram_tensor("z", [R, D], f32, kind="ExternalInput")
    rowmax_dram = nc.dram_tensor("row_max", [P, MB], f32, kind="ExternalOutput")
    pos_dram = nc.dram_tensor("pos", [P, MB], f32, kind="ExternalOutput")

    with tile.TileContext(nc) as tc, ExitStack() as ctx:
        singles = ctx.enter_context(tc.tile_pool(name="singles", bufs=1))
        big = ctx.enter_context(tc.tile_pool(name="big", bufs=1))
        sq_pool = ctx.enter_context(tc.tile_pool(name="sq_pool", bufs=2))
        cand_pool = ctx.enter_context(tc.tile_pool(name="cand_pool", bufs=2))
        fold_pool = ctx.enter_context(tc.tile_pool(name="fold_pool", bufs=2))
        scr_pool = ctx.enter_context(tc.tile_pool(name="scr_pool", bufs=2))
        dram = ctx.enter_context(
            tc.tile_pool(name="dram", bufs=1, space=bass.MemorySpace.DRAM)
        )
        psum = ctx.enter_context(
            tc.tile_pool(name="psum", bufs=2, space=bass.MemorySpace.PSUM)
        )

        # --- constants ---
        ident_f = singles.tile([P, P], f32)
        make_identity(nc, ident_f)
        bigI = singles.tile([P, P], f32)
        nc.gpsimd.memset(bigI, 0.0)
        nc.gpsimd.affine_select(
            out=bigI,
            in_=bigI,
            compare_op=mybir.AluOpType.not_equal,
            fill=BIG,
            base=0,
            pattern=[[-1, P]],
            channel_multiplier=1,
        )

        # --- persistent buffers ---
        zb = big.tile([P, NT_ROW, D], bf16)     # row-major bf16 (scaled in place)
        zT0 = big.tile([P, R], bf16)            # [d 0:128, row]
        zT1 = big.tile([P, R], bf16)            # [d 128:256, row]
        zT = [zT0, zT1]
        n2 = singles.tile([P, NT_ROW], f32)
        nrm = singles.tile([P, NT_ROW], f32)
        inv = singles.tile([P, NT_ROW], f32)
        rowmax_sb = singles.tile([P, MB], f32)
        pos_sb = singles.tile([P, MB], f32)
        znb_d = dram.tile([R, D], bf16)         # DRAM scratch for transpose

        z_src = z_dram.ap().rearrange("(t p) d -> p t d", p=P)

        # --- preprocessing, per chunk of 1024 rows ---
        for g in range(CH):
            gs = slice(g * TPG, (g + 1) * TPG)
            # cast-DMA: DRAM f32 -> SBUF bf16 (SWDGE casts inline)
            nc.gpsimd.dma_start(out=zb[:, gs, :], in_=z_src[:, gs, :])
            sq = sq_pool.tile([P, TPG, D], f32, name="sq")
            nc.scalar.activation(out=sq, in_=zb[:, gs, :], func=AF.Square)
            nc.vector.reduce_sum(out=n2[:, gs], in_=sq, axis=X)
            nc.scalar.activation(out=nrm[:, gs], in_=n2[:, gs], func=AF.Sqrt)
            nc.vector.reciprocal(out=inv[:, gs], in_=nrm[:, gs])
            for j in range(TPG):
                t = g * TPG + j
                nc.vector.tensor_scalar_mul(
                    zb[:, t, :], zb[:, t, :], inv[:, t : t + 1]
                )
            nc.sync.dma_start(
                out=znb_d[g * MROWS : (g + 1) * MROWS, :].rearrange(
                    "(j p) d -> p j d", p=P
                ),
                in_=zb[:, gs, :],
            )
            # xbar transpose loads: [1024, 128] DRAM -> [128, 1024] SBUF
            for c in range(KC):
                eng = nc.sync if c == 0 else nc.scalar
                eng.dma_start(
                    out=zT[c][:, g * MROWS : (g + 1) * MROWS],
                    in_=znb_d[g * MROWS : (g + 1) * MROWS, c * P : (c + 1) * P],
                    transpose=True,
                )

        # --- main: per 128-row block, 4 psum quads over all 8192 columns ---
        for b in range(MB):
            o = b * P
            cand = cand_pool.tile([P, R], bf16, name="cand")
            for q in range(NQ):
                pp = psum.tile([P, QUAD], f32, name="pp")
                for c in range(KC):
                    for u in range(QUAD // 512):
                        col = q * QUAD + u * 512
                        nc.tensor.matmul(
                            pp[:, u * 512 : (u + 1) * 512],
                            zT[c][:, o : o + P],
                            zT[c][:, col : col + 512],
                            start=(c == 0),
                            stop=(c == KC - 1),
                        )
                if q == 0:
                    # mask self-similarity (diag block at columns o..o+128)
                    nc.vector.tensor_sub(
                        pp[:, o : o + P], pp[:, o : o + P], bigI
                    )
                if q == 2:
                    # positives: diag of the block at columns 4096+o
                    scr = scr_pool.tile([P, P], f32, name="scr")
                    nc.vector.tensor_mul(scr, pp[:, o : o + P], ident_f)
                    nc.vector.reduce_sum(out=pos_sb[:, b : b + 1], in_=scr, axis=X)
                # evacuate quad to bf16 candidates (ACT)
                nc.scalar.copy(out=cand[:, q * QUAD : (q + 1) * QUAD], in_=pp[:])
            # deep fold (DVE, bf16 2x) then final reduce
            w = fold_pool.tile([P, R // 2], bf16, name="w")
            nc.vector.tensor_max(w[:], cand[:, : R // 2], cand[:, R // 2 :])
            nc.vector.tensor_max(w[:, :2048], w[:, :2048], w[:, 2048:4096])
            nc.vector.tensor_max(w[:, :1024], w[:, :1024], w[:, 1024:2048])
            nc.vector.tensor_max(w[:, :512], w[:, :512], w[:, 512:1024])
            nc.vector.reduce_max(
                out=rowmax_sb[:, b : b + 1], in_=w[:, :512], axis=X
            )

        nc.sync.dma_start(out=rowmax_dram.ap(), in_=rowmax_sb[:])
        nc.sync.dma_start(out=pos_dram.ap(), in_=pos_sb[:])

    nc.compile()
    return nc


def _get_nc():
    if "nc" not in _CACHE:
        _CACHE["nc"] = _build_nc()
    return _CACHE["nc"]


def _finish(rowmax_all: np.ndarray, pos_all: np.ndarray) -> np.ndarray:
    negmax = rowmax_all.astype(np.float64) / TEMPERATURE
    pos = pos_all.astype(np.float64) / TEMPERATURE
    m = pos.max()
    lse = np.log(np.exp(pos - m).sum()) + m
    return np.array(negmax.mean() - lse, dtype=np.float32)


def kernel(z_i: np.ndarray, z_j: np.ndarray, _collect=None, _run_kwargs=None) -> np.ndarray:
    from concourse.bass_utils import run_bass_kernel_spmd

    z_full = np.concatenate(
        [np.asarray(z_i, np.float32), np.asarray(z_j, np.float32)], axis=0
    )
    in_maps = [
        {"z": np.ascontiguousarray(np.roll(z_full, -k * MROWS, axis=0))}
        for k in range(NCORES)
    ]
    nc = _get_nc()
    res = run_bass_kernel_spmd(
        nc, in_maps, core_ids=list(range(NCORES)), **(_run_kwargs or {})
    )
    if _collect is not None:
        _collect.append(res)
    rowmax_all = np.concatenate(
        [r["row_max"].T.reshape(-1) for r in res.results]
    )  # (8192,) in original row order
    pos_all = np.concatenate([r["pos"].T.reshape(-1) for r in res.results])
    return _finish(rowmax_all, pos_all)
